# revision 36
# baseline (speedup 1.0000x reference)
"""MultiHeadAttention kernel for Trainium2 (8 NeuronCores, data-parallel over batch).

Reference computation (B=8, S=2048, D=64, concat=768):
    q = x @ Wq.T ; k = x @ Wk.T ; v = x @ Wv.T          # [B,S,768]
    scores = (q @ k.T) / sqrt(64)                        # [B,S,S]  (full concat dim!)
    attn = softmax(scores, -1)
    out = (attn @ v) @ Wf.T + b                          # [B,S,64]

Key algebraic identity: since the scores contract over the FULL concat dim,
q @ k.T = x (Wq^T Wk) x^T with A := Wq^T Wk in R^{64x64}; similarly
(attn @ v) @ Wf^T = attn @ (x @ W2) with W2 := Wv^T Wf^T in R^{64x64}.
This cuts the matmul work ~12x. Softmax normalization is folded into the
second matmul by augmenting z = x @ W2 with a ones column (giving row sums),
and dividing at the end (exp without max-subtraction is safe: |scores| < ~1).

Layout: everything contracts over the 64-wide feature axis, so x^T and
y^T = A^T x^T live as [64, S] tiles. Score chunks are computed TRANSPOSED
([key-chunk=128 partitions, query=free]) so the exp output feeds the second
matmul directly. Since K=64 only fills half the PE array, score matmuls for
two key chunks are row-packed into array row-groups 0-63 / 64-127 (x^T/y^T
are duplicated to partitions 64-127 by an SBUF->SBUF DMA).
"""

import sys

sys.path.insert(0, "/opt/trn_rl_repo")

import numpy as np

import concourse.bass as bass
import concourse.tile as tile
from concourse import bacc, mybir
from concourse.bass_utils import run_bass_kernel_spmd

F32 = mybir.dt.float32
F32R = mybir.dt.float32r

B, S, D, C = 8, 2048, 64, 768
NCHUNK = S // 128          # 16 key chunks of 128
NPAIR = NCHUNK // 2        # 8 row-packed chunk pairs
NSUP = S // 512            # 4 query superblocks of 512
NW = C // 128              # 6 weight chunks of 128
SCALING = 0.125            # 1/sqrt(64)


def _build_nc():
    nc = bacc.Bacc("TRN2", target_bir_lowering=False, debug=False)

    x_d = nc.dram_tensor("x", [S, D], F32R, kind="ExternalInput")
    wq_d = nc.dram_tensor("w_q", [C, D], F32R, kind="ExternalInput")
    wk_d = nc.dram_tensor("w_k", [C, D], F32R, kind="ExternalInput")
    wv_d = nc.dram_tensor("w_v", [C, D], F32R, kind="ExternalInput")
    wf_d = nc.dram_tensor("w_final", [D, C], F32R, kind="ExternalInput")
    b_d = nc.dram_tensor("b_final", [D], F32, kind="ExternalInput")
    ident_d = nc.dram_tensor("ident", [128, 128], F32R, kind="ExternalInput")
    ones_d = nc.dram_tensor("ones", [128, NCHUNK], F32R, kind="ExternalInput")
    zeros_d = nc.dram_tensor("zeros", [1, 512], F32R, kind="ExternalInput")
    out_d = nc.dram_tensor("out", [S, D], F32, kind="ExternalOutput")

    with tile.TileContext(nc) as tc:
        _emit(tc, x_d, wq_d, wk_d, wv_d, wf_d, b_d, ident_d, ones_d, zeros_d, out_d)
    nc.compile()
    return nc


def _emit(tc, x_d, wq_d, wk_d, wv_d, wf_d, b_d, ident_d, ones_d, zeros_d, out_d):
    nc = tc.nc
    const = tc.alloc_tile_pool(name="const", bufs=1)

    # ident + x first on the HW DGE (transposes need them earliest);
    # weights/bias/ones go via GpSimd SWDGE in parallel
    ident = const.tile([128, 128], F32R)
    nc.sync.dma_start(ident[:], ident_d.ap())

    # ---- load x as 16 chunks of [128, 64] (4 DMAs so transposes start early)
    x_sb = const.tile([128, NCHUNK, D], F32R)
    x_ap = x_d.ap().rearrange("(n p) d -> p n d", p=128)
    for g in range(4):
        nc.sync.dma_start(x_sb[:, 4 * g : 4 * (g + 1), :], x_ap[:, 4 * g : 4 * (g + 1), :])

    # warm the ACT exp table early so the ~2.7us table load overlaps prep
    warm = const.tile([1, 2], F32)
    nc.scalar.activation(out=warm[:], in_=ident[0:1, 0:2].bitcast(F32),
                         func=mybir.ActivationFunctionType.Exp, scale=1.0)

    # ---- load weights ----
    wq_sb = const.tile([128, NW, D], F32R)
    wk_sb = const.tile([128, NW, D], F32R)
    wv_sb = const.tile([128, NW, D], F32R)
    for w_d, w_sb in ((wq_d, wq_sb), (wk_d, wk_sb), (wv_d, wv_sb)):
        nc.gpsimd.dma_start(w_sb[:], w_d.ap().rearrange("(n p) d -> p n d", p=128))
    wf_sb = const.tile([D, C], F32R)
    nc.sync.dma_start(wf_sb[:], wf_d.ap())

    # persistent ping-pong tiles for the finalize stage, with row 65
    # pre-zeroed once here so no DMA sits on the finalize critical path
    # (rows are padded to 66 partitions: fp32r transposes need an even
    # innermost free count on the destination)
    ot_tiles = [const.tile([D + 2, 512], F32R, name=f"ot{i}") for i in range(2)]
    for i in range(2):
        nc.gpsimd.dma_start(ot_tiles[i][D + 1 : D + 2, :], zeros_d.ap())

    # bias broadcast to all partitions
    b_bcast = const.tile([128, D], F32)
    b_ap = b_d.ap()
    b_src = bass.AP(tensor=b_ap.tensor, offset=b_ap.offset, ap=[[0, 128]] + list(b_ap.ap))
    nc.gpsimd.dma_start(b_bcast[:], b_src)

    # x^T / y^T duplicated on partitions 64-127 for score row-packing
    xTd = const.tile([128, S], F32R)
    yTd = const.tile([128, S], F32R)
    z_sb = const.tile([128, NCHUNK, D + 1], F32R)  # z = x @ W2, col 64 = ones
    nc.gpsimd.dma_start(z_sb[:, :, D : D + 1], ones_d.ap().unsqueeze(2))

    a_sb = const.tile([D, D], F32R)        # A = Wq^T Wk
    w2_sb = const.tile([D, D], F32R)       # W2 = Wv^T Wf^T
    wfT_sb = const.tile([128, NW, D], F32R)

    with tc.tile_pool(name="prep_psA", bufs=1, space="PSUM") as ppsa:
        # Wf^T chunks via PE transpose
        for n in range(NW):
            pt = ppsa.tile([128, D], F32R, tag="wft", bufs=2)
            nc.tensor.transpose(pt[:], wf_sb[:, n * 128 : (n + 1) * 128], ident[0:D, 0:D])
            nc.vector.tensor_copy(wfT_sb[:, n, :], pt[:])

        # A and W2 (accumulate over 6 chunks of the contraction dim C)
        a_ps = ppsa.tile([D, D], F32, tag="aw", bufs=2)
        for n in range(NW):
            nc.tensor.matmul(a_ps[:], wq_sb[:, n, :], wk_sb[:, n, :],
                             start=(n == 0), stop=(n == NW - 1))
        nc.vector.tensor_copy(a_sb[:], a_ps[:])
        w2_ps = ppsa.tile([D, D], F32, tag="aw", bufs=2)
        for n in range(NW):
            nc.tensor.matmul(w2_ps[:], wv_sb[:, n, :], wfT_sb[:, n, :],
                             start=(n == 0), stop=(n == NW - 1))
        nc.vector.tensor_copy(w2_sb[:], w2_ps[:])

    with tc.tile_pool(name="prep_psB", bufs=1, space="PSUM") as pps2:
        # x^T via PE transposes (copies alternate Vector/Scalar), with the
        # z matmul for each chunk (z = x @ W2) interleaved right after.
        # One dup DMA at the end replicates rows 0-63 onto 64-127.
        for n in range(NCHUNK):
            pt = pps2.tile([D, 128], F32R, tag="xt", bufs=4)
            nc.tensor.transpose(pt[:], x_sb[:, n, :], ident[:])
            if n % 2 == 0:
                nc.vector.tensor_copy(xTd[0:D, n * 128 : (n + 1) * 128], pt[:])
            else:
                nc.scalar.copy(xTd[0:D, n * 128 : (n + 1) * 128], pt[:])
            zp = pps2.tile([128, D], F32, tag="z", bufs=2)
            nc.tensor.matmul(zp[:], xTd[0:D, n * 128 : (n + 1) * 128], w2_sb[:],
                             start=True, stop=True)
            nc.vector.tensor_copy(z_sb[:, n, 0:D], zp[:])
        nc.sync.dma_start(xTd[D:128, :], xTd[0:D, :])

        # y^T = A^T @ x^T ; rows 64-127 by one dup DMA at the end
        for j in range(NSUP):
            yp = pps2.tile([D, 512], F32, tag="yt", bufs=2)
            nc.tensor.matmul(yp[:], a_sb[:], xTd[0:D, j * 512 : (j + 1) * 512],
                             start=True, stop=True)
            if j % 2 == 0:
                nc.vector.tensor_copy(yTd[0:D, j * 512 : (j + 1) * 512], yp[:])
            else:
                nc.scalar.copy(yTd[0:D, j * 512 : (j + 1) * 512], yp[:])
        nc.sync.dma_start(yTd[D:128, :], yTd[0:D, :])

    # ---- main loop: row-packed scores^T -> exp -> O' accumulation ----
    with tc.tile_pool(name="oacc", bufs=1, space="PSUM") as oacc_pool:
        o_ps = [oacc_pool.tile([D + 1, 512], F32, tag=f"o{j}", name=f"o_ps{j}")
                for j in range(NSUP)]

        with tc.tile_pool(name="sc_ps", bufs=2, space="PSUM") as scp, \
             tc.tile_pool(name="et", bufs=2) as etp:
            for p in range(NPAIR):
                n0, n1 = 2 * p, 2 * p + 1
                lhs0 = xTd[0:D, n0 * 128 : (n0 + 1) * 128]
                lhs1 = xTd[D:128, n1 * 128 : (n1 + 1) * 128]
                # eT layout: [j-block 4][chunk 2][512]
                eT = etp.tile([128, 2 * S], F32R, tag="et")
                for j in range(NSUP):
                    sc = scp.tile([128, 1024], F32, tag="sc")
                    # two K=64 matmuls packed into PE row groups 0-1 / 2-3
                    nc.tensor.matmul(sc[:, 0:512], lhs0,
                                     yTd[0:D, j * 512 : (j + 1) * 512],
                                     start=True, stop=True)
                    nc.tensor.matmul(sc[:, 512:1024], lhs1,
                                     yTd[D:128, j * 512 : (j + 1) * 512],
                                     start=True, stop=True)
                    nc.scalar.activation(out=eT[:, j * 1024 : (j + 1) * 1024], in_=sc[:],
                                         func=mybir.ActivationFunctionType.Exp,
                                         scale=SCALING)
                for j in range(NSUP):
                    nc.tensor.matmul(o_ps[j][:], z_sb[:, n0, :],
                                     eT[:, j * 1024 : j * 1024 + 512],
                                     start=(p == 0), stop=False)
                    nc.tensor.matmul(o_ps[j][:], z_sb[:, n1, :],
                                     eT[:, j * 1024 + 512 : (j + 1) * 1024],
                                     start=False, stop=(p == NPAIR - 1))

        # ---- finalize: transpose back, normalize, bias, store ----
        with tc.tile_pool(name="fin_ps", bufs=4, space="PSUM") as fps, \
             tc.tile_pool(name="fin_sb", bufs=4) as fsb, \
             tc.tile_pool(name="out_sb", bufs=4) as osb:
            for j in range(NSUP):
                # ping-pong const tiles whose padding row 65 is pre-zeroed;
                # copy split between Vector and Scalar by column halves
                ot = ot_tiles[j % 2]
                nc.vector.tensor_copy(ot[0 : D + 1, 0:256], o_ps[j][:, 0:256])
                nc.scalar.copy(ot[0 : D + 1, 256:512], o_ps[j][:, 256:512])
                for q in range(4):
                    pt = fps.tile([128, D + 2], F32R, tag="fin")
                    nc.tensor.transpose(pt[:], ot[:, q * 128 : (q + 1) * 128],
                                        ident[0 : D + 2, 0 : D + 2])
                    r_sb = fsb.tile([128, 1], F32, tag="r")
                    nc.vector.reciprocal(r_sb[:], pt[:, D : D + 1].bitcast(F32))
                    o_out = osb.tile([128, D], F32, tag="oo")
                    nc.vector.tensor_scalar_mul(o_out[:], pt[:, 0:D], r_sb[:])
                    nc.gpsimd.tensor_add(o_out[:], o_out[:], b_bcast[:])
                    row0 = j * 512 + q * 128
                    nc.sync.dma_start(out_d.ap()[row0 : row0 + 128, :], o_out[:])

    const.release()


_NC_CACHE = {}


def _get_nc():
    if "nc" not in _NC_CACHE:
        _NC_CACHE["nc"] = _build_nc()
    return _NC_CACHE["nc"]


def kernel(x, w_q, w_k, w_v, w_final, b_final, _trace=False):
    nc = _get_nc()
    x = np.ascontiguousarray(np.asarray(x, dtype=np.float32))
    shared = {
        "w_q": np.ascontiguousarray(np.asarray(w_q, dtype=np.float32)),
        "w_k": np.ascontiguousarray(np.asarray(w_k, dtype=np.float32)),
        "w_v": np.ascontiguousarray(np.asarray(w_v, dtype=np.float32)),
        "w_final": np.ascontiguousarray(np.asarray(w_final, dtype=np.float32)),
        "b_final": np.ascontiguousarray(np.asarray(b_final, dtype=np.float32)),
        "ident": np.eye(128, dtype=np.float32),
        "ones": np.ones((128, NCHUNK), dtype=np.float32),
        "zeros": np.zeros((1, 512), dtype=np.float32),
    }
    in_maps = [dict(shared, x=x[b]) for b in range(B)]
    res = run_bass_kernel_spmd(nc, in_maps, core_ids=list(range(B)), trace=_trace)
    out = np.stack([res.results[b]["out"] for b in range(B)], axis=0)
    if _trace:
        return out, res
    return out


# revision 44
# speedup vs baseline: 1.0194x; 1.0194x over previous
"""MultiHeadAttention kernel for Trainium2 (8 NeuronCores, data-parallel over batch).

Reference computation (B=8, S=2048, D=64, concat=768):
    q = x @ Wq.T ; k = x @ Wk.T ; v = x @ Wv.T          # [B,S,768]
    scores = (q @ k.T) / sqrt(64)                        # [B,S,S]  (full concat dim!)
    attn = softmax(scores, -1)
    out = (attn @ v) @ Wf.T + b                          # [B,S,64]

Key algebraic identity: since the scores contract over the FULL concat dim,
q @ k.T = x (Wq^T Wk) x^T with A := Wq^T Wk in R^{64x64}; similarly
(attn @ v) @ Wf^T = attn @ (x @ W2) with W2 := Wv^T Wf^T in R^{64x64}.
This cuts the matmul work ~12x. Softmax normalization is folded into the
second matmul by augmenting z = x @ W2 with a ones column (giving row sums),
and dividing at the end (exp without max-subtraction is safe: |scores| < ~1).

Layout: everything contracts over the 64-wide feature axis, so x^T and
y^T = A^T x^T live as [64, S] tiles. Score chunks are computed TRANSPOSED
([key-chunk=128 partitions, query=free]) so the exp output feeds the second
matmul directly. Since K=64 only fills half the PE array, score matmuls for
two key chunks are row-packed into array row-groups 0-63 / 64-127 (x^T/y^T
are duplicated to partitions 64-127 by an SBUF->SBUF DMA).
"""

import sys

sys.path.insert(0, "/opt/trn_rl_repo")

import numpy as np

import concourse.bass as bass
import concourse.tile as tile
from concourse import bacc, mybir
from concourse.bass_utils import run_bass_kernel_spmd

F32 = mybir.dt.float32
F32R = mybir.dt.float32r

B, S, D, C = 8, 2048, 64, 768
NCHUNK = S // 128          # 16 key chunks of 128
NPAIR = NCHUNK // 2        # 8 row-packed chunk pairs
NSUP = S // 512            # 4 query superblocks of 512
NW = C // 128              # 6 weight chunks of 128
SCALING = 0.125            # 1/sqrt(64)


def _build_nc():
    nc = bacc.Bacc("TRN2", target_bir_lowering=False, debug=False)

    x_d = nc.dram_tensor("x", [S, D], F32R, kind="ExternalInput")
    wq_d = nc.dram_tensor("w_q", [C, D], F32R, kind="ExternalInput")
    wk_d = nc.dram_tensor("w_k", [C, D], F32R, kind="ExternalInput")
    wv_d = nc.dram_tensor("w_v", [C, D], F32R, kind="ExternalInput")
    wf_d = nc.dram_tensor("w_final", [D, C], F32R, kind="ExternalInput")
    b_d = nc.dram_tensor("b_final", [D], F32, kind="ExternalInput")
    ident_d = nc.dram_tensor("ident", [128, 128], F32R, kind="ExternalInput")
    ones_d = nc.dram_tensor("ones", [128, NCHUNK], F32R, kind="ExternalInput")
    zeros_d = nc.dram_tensor("zeros", [1, 512], F32R, kind="ExternalInput")
    out_d = nc.dram_tensor("out", [S, D], F32, kind="ExternalOutput")

    with tile.TileContext(nc) as tc:
        _emit(tc, x_d, wq_d, wk_d, wv_d, wf_d, b_d, ident_d, ones_d, zeros_d, out_d)
    nc.compile()
    return nc


def _emit(tc, x_d, wq_d, wk_d, wv_d, wf_d, b_d, ident_d, ones_d, zeros_d, out_d):
    nc = tc.nc
    const = tc.alloc_tile_pool(name="const", bufs=1)

    # ident + x first on the HW DGE (transposes need them earliest);
    # weights/bias/ones go via GpSimd SWDGE in parallel
    ident = const.tile([128, 128], F32R)
    nc.sync.dma_start(ident[:], ident_d.ap())

    # ---- load x as 16 chunks of [128, 64] (4 DMAs so transposes start early)
    x_sb = const.tile([128, NCHUNK, D], F32R)
    x_ap = x_d.ap().rearrange("(n p) d -> p n d", p=128)
    for g in range(4):
        nc.sync.dma_start(x_sb[:, 4 * g : 4 * (g + 1), :], x_ap[:, 4 * g : 4 * (g + 1), :])

    # warm the ACT exp table early so the ~2.7us table load overlaps prep,
    # and issue a tiny PE op so the PE instruction-stream IRAM fetch (~3.5us)
    # also happens during the DMA phase instead of before the first real matmul
    warm = const.tile([1, 2], F32)
    nc.scalar.activation(out=warm[:], in_=ident[0:1, 0:2].bitcast(F32),
                         func=mybir.ActivationFunctionType.Exp, scale=1.0)

    # ---- load weights (GpSimd SWDGE, parallel with the sync-queue loads;
    # wf first since the Wf^T -> W2 chain is the longest) ----
    wf_sb = const.tile([D, C], F32R)
    nc.gpsimd.dma_start(wf_sb[:], wf_d.ap())
    wq_sb = const.tile([128, NW, D], F32R)
    wk_sb = const.tile([128, NW, D], F32R)
    wv_sb = const.tile([128, NW, D], F32R)
    for w_d, w_sb in ((wq_d, wq_sb), (wk_d, wk_sb), (wv_d, wv_sb)):
        nc.gpsimd.dma_start(w_sb[:], w_d.ap().rearrange("(n p) d -> p n d", p=128))

    # persistent ping-pong tiles for the finalize stage, with row 65
    # pre-zeroed once here so no DMA sits on the finalize critical path
    # (rows are padded to 66 partitions: fp32r transposes need an even
    # innermost free count on the destination)
    ot_tiles = [const.tile([D + 2, 512], F32R, name=f"ot{i}") for i in range(2)]
    for i in range(2):
        nc.gpsimd.dma_start(ot_tiles[i][D + 1 : D + 2, :], zeros_d.ap())

    # bias broadcast to all partitions
    b_bcast = const.tile([128, D], F32)
    b_ap = b_d.ap()
    b_src = bass.AP(tensor=b_ap.tensor, offset=b_ap.offset, ap=[[0, 128]] + list(b_ap.ap))
    nc.gpsimd.dma_start(b_bcast[:], b_src)

    # x^T / y^T duplicated on partitions 64-127 for score row-packing
    xTd = const.tile([128, S], F32R)
    yTd = const.tile([128, S], F32R)
    z_sb = const.tile([128, NCHUNK, D + 1], F32R)  # z = x @ W2, col 64 = ones
    nc.gpsimd.dma_start(z_sb[:, :, D : D + 1], ones_d.ap().unsqueeze(2))

    a_sb = const.tile([D, D], F32R)        # A = Wq^T Wk
    w2_sb = const.tile([D, D], F32R)       # W2 = Wv^T Wf^T
    wfT_sb = const.tile([128, NW, D], F32R)

    with tc.tile_pool(name="prep_psA", bufs=1, space="PSUM") as ppsa:
        pe_warm = ppsa.tile([32, 32], F32R, tag="warm", bufs=1)
        nc.tensor.transpose(pe_warm[:], ident[0:32, 0:32], ident[0:32, 0:32])

        # Wf^T chunks via PE transpose
        for n in range(NW):
            pt = ppsa.tile([128, D], F32R, tag="wft", bufs=2)
            nc.tensor.transpose(pt[:], wf_sb[:, n * 128 : (n + 1) * 128], ident[0:D, 0:D])
            nc.vector.tensor_copy(wfT_sb[:, n, :], pt[:])

        # A and W2 (accumulate over 6 chunks of the contraction dim C)
        a_ps = ppsa.tile([D, D], F32, tag="aw", bufs=2)
        for n in range(NW):
            nc.tensor.matmul(a_ps[:], wq_sb[:, n, :], wk_sb[:, n, :],
                             start=(n == 0), stop=(n == NW - 1))
        nc.vector.tensor_copy(a_sb[:], a_ps[:])
        w2_ps = ppsa.tile([D, D], F32, tag="aw", bufs=2)
        for n in range(NW):
            nc.tensor.matmul(w2_ps[:], wv_sb[:, n, :], wfT_sb[:, n, :],
                             start=(n == 0), stop=(n == NW - 1))
        nc.vector.tensor_copy(w2_sb[:], w2_ps[:])

    with tc.tile_pool(name="prep_psB", bufs=1, space="PSUM") as pps2:
        # x^T via PE transposes (copies alternate Vector/Scalar); one dup
        # DMA at the end replicates rows 0-63 onto partitions 64-127
        for n in range(NCHUNK):
            pt = pps2.tile([D, 128], F32R, tag="xt", bufs=4)
            nc.tensor.transpose(pt[:], x_sb[:, n, :], ident[:])
            if n % 2 == 0:
                nc.vector.tensor_copy(xTd[0:D, n * 128 : (n + 1) * 128], pt[:])
            else:
                nc.scalar.copy(xTd[0:D, n * 128 : (n + 1) * 128], pt[:])
        nc.sync.dma_start(xTd[D:128, :], xTd[0:D, :])

        # y^T = A^T @ x^T ; rows 64-127 by one dup DMA at the end
        for j in range(NSUP):
            yp = pps2.tile([D, 512], F32, tag="yt", bufs=2)
            nc.tensor.matmul(yp[:], a_sb[:], xTd[0:D, j * 512 : (j + 1) * 512],
                             start=True, stop=True)
            if j % 2 == 0:
                nc.vector.tensor_copy(yTd[0:D, j * 512 : (j + 1) * 512], yp[:])
            else:
                nc.scalar.copy(yTd[0:D, j * 512 : (j + 1) * 512], yp[:])
        nc.sync.dma_start(yTd[D:128, :], yTd[0:D, :])

        # z chunks: z[t, :] = x[t, :] @ W2, row-packed two at a time
        # (K=64 pairs in PE row groups, like the score matmuls)
        w2d = const.tile([128, D], F32R)
        nc.sync.dma_start(w2d[D:128, :], w2_sb[:])
        for h in range(NPAIR):
            n0, n1 = 2 * h, 2 * h + 1
            zp0 = pps2.tile([128, D], F32, tag="z0", bufs=1)
            zp1 = pps2.tile([128, D], F32, tag="z1", bufs=1)
            nc.tensor.matmul(zp0[:], xTd[0:D, n0 * 128 : (n0 + 1) * 128], w2_sb[:],
                             start=True, stop=True)
            nc.tensor.matmul(zp1[:], xTd[D:128, n1 * 128 : (n1 + 1) * 128],
                             w2d[D:128, :], start=True, stop=True)
            nc.vector.tensor_copy(z_sb[:, n0, 0:D], zp0[:])
            nc.scalar.copy(z_sb[:, n1, 0:D], zp1[:])

    # ---- main loop: row-packed scores^T -> exp -> O' accumulation ----
    with tc.tile_pool(name="oacc", bufs=1, space="PSUM") as oacc_pool:
        o_ps = [oacc_pool.tile([D + 1, 512], F32, tag=f"o{j}", name=f"o_ps{j}")
                for j in range(NSUP)]

        with tc.tile_pool(name="sc_ps", bufs=2, space="PSUM") as scp, \
             tc.tile_pool(name="et", bufs=2) as etp:
            for p in range(NPAIR):
                n0, n1 = 2 * p, 2 * p + 1
                lhs0 = xTd[0:D, n0 * 128 : (n0 + 1) * 128]
                lhs1 = xTd[D:128, n1 * 128 : (n1 + 1) * 128]
                # eT layout: [j-block 4][chunk 2][512]
                eT = etp.tile([128, 2 * S], F32R, tag="et")
                for j in range(NSUP):
                    sc = scp.tile([128, 1024], F32, tag="sc")
                    # two K=64 matmuls packed into PE row groups 0-1 / 2-3
                    nc.tensor.matmul(sc[:, 0:512], lhs0,
                                     yTd[0:D, j * 512 : (j + 1) * 512],
                                     start=True, stop=True)
                    nc.tensor.matmul(sc[:, 512:1024], lhs1,
                                     yTd[D:128, j * 512 : (j + 1) * 512],
                                     start=True, stop=True)
                    nc.scalar.activation(out=eT[:, j * 1024 : (j + 1) * 1024], in_=sc[:],
                                         func=mybir.ActivationFunctionType.Exp,
                                         scale=SCALING)
                for j in range(NSUP):
                    nc.tensor.matmul(o_ps[j][:], z_sb[:, n0, :],
                                     eT[:, j * 1024 : j * 1024 + 512],
                                     start=(p == 0), stop=False)
                    nc.tensor.matmul(o_ps[j][:], z_sb[:, n1, :],
                                     eT[:, j * 1024 + 512 : (j + 1) * 1024],
                                     start=False, stop=(p == NPAIR - 1))

        # ---- finalize: transpose back, normalize, bias, store ----
        with tc.tile_pool(name="fin_ps", bufs=4, space="PSUM") as fps, \
             tc.tile_pool(name="fin_sb", bufs=4) as fsb, \
             tc.tile_pool(name="out_sb", bufs=4) as osb:
            for j in range(NSUP):
                # ping-pong const tiles whose padding row 65 is pre-zeroed;
                # copy split between Vector and Scalar by column halves
                ot = ot_tiles[j % 2]
                nc.vector.tensor_copy(ot[0 : D + 1, 0:256], o_ps[j][:, 0:256])
                nc.scalar.copy(ot[0 : D + 1, 256:512], o_ps[j][:, 256:512])
                for q in range(4):
                    pt = fps.tile([128, D + 2], F32R, tag="fin")
                    nc.tensor.transpose(pt[:], ot[:, q * 128 : (q + 1) * 128],
                                        ident[0 : D + 2, 0 : D + 2])
                    r_sb = fsb.tile([128, 1], F32, tag="r")
                    nc.vector.reciprocal(r_sb[:], pt[:, D : D + 1].bitcast(F32))
                    o_out = osb.tile([128, D], F32, tag="oo")
                    nc.vector.tensor_scalar_mul(o_out[:], pt[:, 0:D], r_sb[:])
                    nc.gpsimd.tensor_add(o_out[:], o_out[:], b_bcast[:])
                    row0 = j * 512 + q * 128
                    nc.sync.dma_start(out_d.ap()[row0 : row0 + 128, :], o_out[:])

    const.release()


_NC_CACHE = {}


def _get_nc():
    if "nc" not in _NC_CACHE:
        _NC_CACHE["nc"] = _build_nc()
    return _NC_CACHE["nc"]


def kernel(x, w_q, w_k, w_v, w_final, b_final, _trace=False):
    nc = _get_nc()
    x = np.ascontiguousarray(np.asarray(x, dtype=np.float32))
    shared = {
        "w_q": np.ascontiguousarray(np.asarray(w_q, dtype=np.float32)),
        "w_k": np.ascontiguousarray(np.asarray(w_k, dtype=np.float32)),
        "w_v": np.ascontiguousarray(np.asarray(w_v, dtype=np.float32)),
        "w_final": np.ascontiguousarray(np.asarray(w_final, dtype=np.float32)),
        "b_final": np.ascontiguousarray(np.asarray(b_final, dtype=np.float32)),
        "ident": np.eye(128, dtype=np.float32),
        "ones": np.ones((128, NCHUNK), dtype=np.float32),
        "zeros": np.zeros((1, 512), dtype=np.float32),
    }
    in_maps = [dict(shared, x=x[b]) for b in range(B)]
    res = run_bass_kernel_spmd(nc, in_maps, core_ids=list(range(B)), trace=_trace)
    out = np.stack([res.results[b]["out"] for b in range(B)], axis=0)
    if _trace:
        return out, res
    return out


# revision 46
# speedup vs baseline: 1.1706x; 1.1483x over previous
"""MultiHeadAttention kernel for Trainium2 (8 NeuronCores, data-parallel over batch).

Reference computation (B=8, S=2048, D=64, concat=768):
    q = x @ Wq.T ; k = x @ Wk.T ; v = x @ Wv.T          # [B,S,768]
    scores = (q @ k.T) / sqrt(64)                        # [B,S,S]  (full concat dim!)
    attn = softmax(scores, -1)
    out = (attn @ v) @ Wf.T + b                          # [B,S,64]

Key algebraic identity: since the scores contract over the FULL concat dim,
q @ k.T = x (Wq^T Wk) x^T with A := Wq^T Wk in R^{64x64}; similarly
(attn @ v) @ Wf^T = attn @ (x @ W2) with W2 := Wv^T Wf^T in R^{64x64}.
This cuts the matmul work ~12x. Softmax normalization is folded into the
second matmul by augmenting z = x @ W2 with a ones column (giving row sums),
and dividing at the end (exp without max-subtraction is safe: |scores| < ~1).

Layout: everything contracts over the 64-wide feature axis, so x^T and
y^T = A^T x^T live as [64, S] tiles. Score chunks are computed TRANSPOSED
([key-chunk=128 partitions, query=free]) so the exp output feeds the second
matmul directly. Since K=64 only fills half the PE array, score matmuls for
two key chunks are row-packed into array row-groups 0-63 / 64-127 (x^T/y^T
are duplicated to partitions 64-127 by an SBUF->SBUF DMA).
"""

import sys

sys.path.insert(0, "/opt/trn_rl_repo")

import numpy as np

import concourse.bass as bass
import concourse.tile as tile
from concourse import bacc, mybir
from concourse.bass_utils import run_bass_kernel_spmd

F32 = mybir.dt.float32
F32R = mybir.dt.float32r

B, S, D, C = 8, 2048, 64, 768
NCHUNK = S // 128          # 16 key chunks of 128
NPAIR = NCHUNK // 2        # 8 row-packed chunk pairs
NSUP = S // 512            # 4 query superblocks of 512
NW = C // 128              # 6 weight chunks of 128
SCALING = 0.125            # 1/sqrt(64)


def _build_nc():
    nc = bacc.Bacc("TRN2", target_bir_lowering=False, debug=False)

    x_d = nc.dram_tensor("x", [S, D], F32R, kind="ExternalInput")
    wq_d = nc.dram_tensor("w_q", [C, D], F32R, kind="ExternalInput")
    wk_d = nc.dram_tensor("w_k", [C, D], F32R, kind="ExternalInput")
    wv_d = nc.dram_tensor("w_v", [C, D], F32R, kind="ExternalInput")
    wf_d = nc.dram_tensor("w_final", [D, C], F32R, kind="ExternalInput")
    b_d = nc.dram_tensor("b_final", [D], F32, kind="ExternalInput")
    ident_d = nc.dram_tensor("ident", [128, 128], F32R, kind="ExternalInput")
    ones_d = nc.dram_tensor("ones", [128, NCHUNK], F32R, kind="ExternalInput")
    zeros_d = nc.dram_tensor("zeros", [1, 512], F32R, kind="ExternalInput")
    out_d = nc.dram_tensor("out", [S, D], F32, kind="ExternalOutput")

    with tile.TileContext(nc) as tc:
        _emit(tc, x_d, wq_d, wk_d, wv_d, wf_d, b_d, ident_d, ones_d, zeros_d, out_d)
    nc.compile()
    return nc


def _emit(tc, x_d, wq_d, wk_d, wv_d, wf_d, b_d, ident_d, ones_d, zeros_d, out_d):
    nc = tc.nc
    const = tc.alloc_tile_pool(name="const", bufs=1)

    # ident + x first on the HW DGE (transposes need them earliest);
    # weights/bias/ones go via GpSimd SWDGE in parallel
    ident = const.tile([128, 128], F32R)
    nc.sync.dma_start(ident[:], ident_d.ap())

    # ---- load x as 16 chunks of [128, 64] (4 DMAs so transposes start early)
    x_sb = const.tile([128, NCHUNK, D], F32R)
    x_ap = x_d.ap().rearrange("(n p) d -> p n d", p=128)
    for g in range(4):
        nc.sync.dma_start(x_sb[:, 4 * g : 4 * (g + 1), :], x_ap[:, 4 * g : 4 * (g + 1), :])

    # warm the ACT exp table early so the ~2.7us table load overlaps prep,
    # and issue a tiny PE op so the PE instruction-stream IRAM fetch (~3.5us)
    # also happens during the DMA phase instead of before the first real matmul
    warm = const.tile([1, 2], F32)
    nc.scalar.activation(out=warm[:], in_=ident[0:1, 0:2].bitcast(F32),
                         func=mybir.ActivationFunctionType.Exp, scale=1.0)

    # ---- load weights (GpSimd SWDGE, parallel with the sync-queue loads;
    # wf first since the Wf^T -> W2 chain is the longest) ----
    wf_sb = const.tile([D, C], F32R)
    nc.gpsimd.dma_start(wf_sb[:], wf_d.ap())
    wq_sb = const.tile([128, NW, D], F32R)
    wk_sb = const.tile([128, NW, D], F32R)
    wv_sb = const.tile([128, NW, D], F32R)
    for w_d, w_sb in ((wq_d, wq_sb), (wk_d, wk_sb), (wv_d, wv_sb)):
        nc.gpsimd.dma_start(w_sb[:], w_d.ap().rearrange("(n p) d -> p n d", p=128))

    # persistent ping-pong tiles for the finalize stage, with row 65
    # pre-zeroed once here so no DMA sits on the finalize critical path
    # (rows are padded to 66 partitions: fp32r transposes need an even
    # innermost free count on the destination)
    ot_tiles = [const.tile([D + 2, 512], F32R, name=f"ot{i}") for i in range(2)]
    for i in range(2):
        nc.gpsimd.dma_start(ot_tiles[i][D + 1 : D + 2, :], zeros_d.ap())

    # bias broadcast to all partitions
    b_bcast = const.tile([128, D], F32)
    b_ap = b_d.ap()
    b_src = bass.AP(tensor=b_ap.tensor, offset=b_ap.offset, ap=[[0, 128]] + list(b_ap.ap))
    nc.gpsimd.dma_start(b_bcast[:], b_src)

    # x^T / y^T duplicated on partitions 64-127 for score row-packing
    xTd = const.tile([128, S], F32R)
    yTd = const.tile([128, S], F32R)
    z_sb = const.tile([128, NCHUNK, D + 1], F32R)  # z = x @ W2, col 64 = ones
    nc.gpsimd.dma_start(z_sb[:, :, D : D + 1], ones_d.ap().unsqueeze(2))

    a_sb = const.tile([D, D], F32R)        # A = Wq^T Wk
    w2_sb = const.tile([D, D], F32R)       # W2 = Wv^T Wf^T
    wfT_sb = const.tile([128, NW, D], F32R)

    # Single prep PSUM pool; phases that never overlap share tags so the
    # total stays within 7 banks. Emission order puts the x^T transposes
    # first — they gate the whole main loop.
    with tc.tile_pool(name="prep_ps", bufs=1, space="PSUM") as pps:
        pe_warm = pps.tile([32, 32], F32R, tag="warm", bufs=1)
        nc.tensor.transpose(pe_warm[:], ident[0:32, 0:32], ident[0:32, 0:32])

        # x^T via PE transposes (copies alternate Vector/Scalar); one dup
        # DMA at the end replicates rows 0-63 onto partitions 64-127
        for n in range(NCHUNK):
            pt = pps.tile([D, 128], F32R, tag="t0", bufs=3)
            nc.tensor.transpose(pt[:], x_sb[:, n, :], ident[:])
            if n % 2 == 0:
                nc.vector.tensor_copy(xTd[0:D, n * 128 : (n + 1) * 128], pt[:])
            else:
                nc.scalar.copy(xTd[0:D, n * 128 : (n + 1) * 128], pt[:])
        nc.sync.dma_start(xTd[D:128, :], xTd[0:D, :])

        # Wf^T chunks via PE transpose
        for n in range(NW):
            pt = pps.tile([128, D], F32R, tag="t1", bufs=2)
            nc.tensor.transpose(pt[:], wf_sb[:, n * 128 : (n + 1) * 128], ident[0:D, 0:D])
            nc.vector.tensor_copy(wfT_sb[:, n, :], pt[:])

        # A = Wq^T Wk (accumulate over 6 chunks of the contraction dim C)
        a_ps = pps.tile([D, D], F32, tag="t2", bufs=2)
        for n in range(NW):
            nc.tensor.matmul(a_ps[:], wq_sb[:, n, :], wk_sb[:, n, :],
                             start=(n == 0), stop=(n == NW - 1))
        nc.vector.tensor_copy(a_sb[:], a_ps[:])

        # y^T = A^T @ x^T ; rows 64-127 by one dup DMA at the end
        for j in range(NSUP):
            yp = pps.tile([D, 512], F32, tag="t2", bufs=2)
            nc.tensor.matmul(yp[:], a_sb[:], xTd[0:D, j * 512 : (j + 1) * 512],
                             start=True, stop=True)
            if j % 2 == 0:
                nc.vector.tensor_copy(yTd[0:D, j * 512 : (j + 1) * 512], yp[:])
            else:
                nc.scalar.copy(yTd[0:D, j * 512 : (j + 1) * 512], yp[:])
        nc.sync.dma_start(yTd[D:128, :], yTd[0:D, :])

        # W2 = Wv^T Wf^T
        w2_ps = pps.tile([D, D], F32, tag="t2", bufs=2)
        for n in range(NW):
            nc.tensor.matmul(w2_ps[:], wv_sb[:, n, :], wfT_sb[:, n, :],
                             start=(n == 0), stop=(n == NW - 1))
        nc.vector.tensor_copy(w2_sb[:], w2_ps[:])

        # z chunks: z[t, :] = x[t, :] @ W2, row-packed two at a time
        # (K=64 pairs in PE row groups, like the score matmuls)
        w2d = const.tile([128, D], F32R)
        nc.sync.dma_start(w2d[D:128, :], w2_sb[:])
        for h in range(NPAIR):
            n0, n1 = 2 * h, 2 * h + 1
            zp0 = pps.tile([128, D], F32, tag="t0", bufs=3)
            zp1 = pps.tile([128, D], F32, tag="t1", bufs=2)
            nc.tensor.matmul(zp0[:], xTd[0:D, n0 * 128 : (n0 + 1) * 128], w2_sb[:],
                             start=True, stop=True)
            nc.tensor.matmul(zp1[:], xTd[D:128, n1 * 128 : (n1 + 1) * 128],
                             w2d[D:128, :], start=True, stop=True)
            nc.vector.tensor_copy(z_sb[:, n0, 0:D], zp0[:])
            nc.scalar.copy(z_sb[:, n1, 0:D], zp1[:])

    # ---- main loop: row-packed scores^T -> exp -> O' accumulation ----
    with tc.tile_pool(name="oacc", bufs=1, space="PSUM") as oacc_pool:
        o_ps = [oacc_pool.tile([D + 1, 512], F32, tag=f"o{j}", name=f"o_ps{j}")
                for j in range(NSUP)]

        with tc.tile_pool(name="sc_ps", bufs=2, space="PSUM") as scp, \
             tc.tile_pool(name="et", bufs=2) as etp:
            for p in range(NPAIR):
                n0, n1 = 2 * p, 2 * p + 1
                lhs0 = xTd[0:D, n0 * 128 : (n0 + 1) * 128]
                lhs1 = xTd[D:128, n1 * 128 : (n1 + 1) * 128]
                # eT layout: [j-block 4][chunk 2][512]
                eT = etp.tile([128, 2 * S], F32R, tag="et")
                for j in range(NSUP):
                    sc = scp.tile([128, 1024], F32, tag="sc")
                    # two K=64 matmuls packed into PE row groups 0-1 / 2-3
                    nc.tensor.matmul(sc[:, 0:512], lhs0,
                                     yTd[0:D, j * 512 : (j + 1) * 512],
                                     start=True, stop=True)
                    nc.tensor.matmul(sc[:, 512:1024], lhs1,
                                     yTd[D:128, j * 512 : (j + 1) * 512],
                                     start=True, stop=True)
                    nc.scalar.activation(out=eT[:, j * 1024 : (j + 1) * 1024], in_=sc[:],
                                         func=mybir.ActivationFunctionType.Exp,
                                         scale=SCALING)
                for j in range(NSUP):
                    nc.tensor.matmul(o_ps[j][:], z_sb[:, n0, :],
                                     eT[:, j * 1024 : j * 1024 + 512],
                                     start=(p == 0), stop=False)
                    nc.tensor.matmul(o_ps[j][:], z_sb[:, n1, :],
                                     eT[:, j * 1024 + 512 : (j + 1) * 1024],
                                     start=False, stop=(p == NPAIR - 1))

        # ---- finalize: transpose back, normalize, bias, store.
        # All 4 query-128-chunks of a superblock land in ONE psum tile so the
        # reciprocal / multiply / bias-add run as single wide ops (DVE DRAIN
        # overhead is per-instruction, so fewer, fatter ops win).
        with tc.tile_pool(name="fin_ps", bufs=2, space="PSUM") as fps, \
             tc.tile_pool(name="fin_sb", bufs=2) as fsb, \
             tc.tile_pool(name="out_sb", bufs=2) as osb:
            out_r = out_d.ap().rearrange("(j q p) d -> j p q d", p=128, q=4)
            for j in range(NSUP):
                # ping-pong const tiles whose padding row 65 is pre-zeroed;
                # copy split between Vector and Scalar by column halves
                ot = ot_tiles[j % 2]
                nc.vector.tensor_copy(ot[0 : D + 1, 0:256], o_ps[j][:, 0:256])
                nc.scalar.copy(ot[0 : D + 1, 256:512], o_ps[j][:, 256:512])
                pt = fps.tile([128, 4, D + 2], F32R, tag="fin")
                for q in range(4):
                    nc.tensor.transpose(pt[:, q, :], ot[:, q * 128 : (q + 1) * 128],
                                        ident[0 : D + 2, 0 : D + 2])
                r_sb = fsb.tile([128, 4], F32, tag="r")
                nc.vector.reciprocal(r_sb[:], pt[:, :, D : D + 1].bitcast(F32))
                o_out = osb.tile([128, 4, D], F32, tag="oo")
                nc.vector.tensor_mul(o_out[:], pt[:, :, 0:D],
                                     r_sb[:].unsqueeze(2).broadcast_to([128, 4, D]))
                nc.gpsimd.tensor_add(
                    o_out[:], o_out[:],
                    b_bcast[:].unsqueeze(1).broadcast_to([128, 4, D]))
                nc.sync.dma_start(out_r[j], o_out[:])

    const.release()


_NC_CACHE = {}


def _get_nc():
    if "nc" not in _NC_CACHE:
        _NC_CACHE["nc"] = _build_nc()
    return _NC_CACHE["nc"]


def kernel(x, w_q, w_k, w_v, w_final, b_final, _trace=False):
    nc = _get_nc()
    x = np.ascontiguousarray(np.asarray(x, dtype=np.float32))
    shared = {
        "w_q": np.ascontiguousarray(np.asarray(w_q, dtype=np.float32)),
        "w_k": np.ascontiguousarray(np.asarray(w_k, dtype=np.float32)),
        "w_v": np.ascontiguousarray(np.asarray(w_v, dtype=np.float32)),
        "w_final": np.ascontiguousarray(np.asarray(w_final, dtype=np.float32)),
        "b_final": np.ascontiguousarray(np.asarray(b_final, dtype=np.float32)),
        "ident": np.eye(128, dtype=np.float32),
        "ones": np.ones((128, NCHUNK), dtype=np.float32),
        "zeros": np.zeros((1, 512), dtype=np.float32),
    }
    in_maps = [dict(shared, x=x[b]) for b in range(B)]
    res = run_bass_kernel_spmd(nc, in_maps, core_ids=list(range(B)), trace=_trace)
    out = np.stack([res.results[b]["out"] for b in range(B)], axis=0)
    if _trace:
        return out, res
    return out


# revision 49
# speedup vs baseline: 1.1914x; 1.0177x over previous
"""MultiHeadAttention kernel for Trainium2 (8 NeuronCores, data-parallel over batch).

Reference computation (B=8, S=2048, D=64, concat=768):
    q = x @ Wq.T ; k = x @ Wk.T ; v = x @ Wv.T          # [B,S,768]
    scores = (q @ k.T) / sqrt(64)                        # [B,S,S]  (full concat dim!)
    attn = softmax(scores, -1)
    out = (attn @ v) @ Wf.T + b                          # [B,S,64]

Key algebraic identity: since the scores contract over the FULL concat dim,
q @ k.T = x (Wq^T Wk) x^T with A := Wq^T Wk in R^{64x64}; similarly
(attn @ v) @ Wf^T = attn @ (x @ W2) with W2 := Wv^T Wf^T in R^{64x64}.
This cuts the matmul work ~12x. Softmax normalization is folded into the
second matmul by augmenting z = x @ W2 with a ones column (giving row sums),
and dividing at the end (exp without max-subtraction is safe: |scores| < ~1).

Layout: everything contracts over the 64-wide feature axis, so x^T and
y^T = A^T x^T live as [64, S] tiles. Score chunks are computed TRANSPOSED
([key-chunk=128 partitions, query=free]) so the exp output feeds the second
matmul directly. Since K=64 only fills half the PE array, score matmuls for
two key chunks are row-packed into array row-groups 0-63 / 64-127 (x^T/y^T
are duplicated to partitions 64-127 by an SBUF->SBUF DMA).
"""

import sys

sys.path.insert(0, "/opt/trn_rl_repo")

import numpy as np

import concourse.bass as bass
import concourse.tile as tile
from concourse import bacc, mybir
from concourse.bass_utils import run_bass_kernel_spmd

F32 = mybir.dt.float32
F32R = mybir.dt.float32r

B, S, D, C = 8, 2048, 64, 768
NCHUNK = S // 128          # 16 key chunks of 128
NPAIR = NCHUNK // 2        # 8 row-packed chunk pairs
NSUP = S // 512            # 4 query superblocks of 512
NW = C // 128              # 6 weight chunks of 128
SCALING = 0.125            # 1/sqrt(64)


def _build_nc():
    nc = bacc.Bacc("TRN2", target_bir_lowering=False, debug=False)

    x_d = nc.dram_tensor("x", [S, D], F32R, kind="ExternalInput")
    wq_d = nc.dram_tensor("w_q", [C, D], F32R, kind="ExternalInput")
    wk_d = nc.dram_tensor("w_k", [C, D], F32R, kind="ExternalInput")
    wv_d = nc.dram_tensor("w_v", [C, D], F32R, kind="ExternalInput")
    wf_d = nc.dram_tensor("w_final", [D, C], F32R, kind="ExternalInput")
    b_d = nc.dram_tensor("b_final", [D], F32, kind="ExternalInput")
    ident_d = nc.dram_tensor("ident", [128, 128], F32R, kind="ExternalInput")
    ones_d = nc.dram_tensor("ones", [128, NCHUNK], F32R, kind="ExternalInput")
    zeros_d = nc.dram_tensor("zeros", [1, 512], F32R, kind="ExternalInput")
    out_d = nc.dram_tensor("out", [S, D], F32, kind="ExternalOutput")

    with tile.TileContext(nc) as tc:
        _emit(tc, x_d, wq_d, wk_d, wv_d, wf_d, b_d, ident_d, ones_d, zeros_d, out_d)
    nc.compile()
    return nc


def _emit(tc, x_d, wq_d, wk_d, wv_d, wf_d, b_d, ident_d, ones_d, zeros_d, out_d):
    nc = tc.nc
    const = tc.alloc_tile_pool(name="const", bufs=1)

    # dep-free first PE instruction: triggers the PE IRAM instruction fetch
    # (~3us) at t=0 instead of after the first operand DMA lands
    nc.tensor.nop(nofuse=True)

    # ident + x first on the HW DGE (transposes need them earliest);
    # weights/bias/ones go via GpSimd SWDGE in parallel
    ident = const.tile([128, 128], F32R)
    nc.sync.dma_start(ident[:], ident_d.ap())

    # ---- load x as 16 chunks of [128, 64] (4 DMAs so transposes start early)
    x_sb = const.tile([128, NCHUNK, D], F32R)
    x_ap = x_d.ap().rearrange("(n p) d -> p n d", p=128)
    for g in range(4):
        nc.sync.dma_start(x_sb[:, 4 * g : 4 * (g + 1), :], x_ap[:, 4 * g : 4 * (g + 1), :])

    # warm the ACT exp table early so the ~2.7us table load overlaps prep,
    # and issue a tiny PE op so the PE instruction-stream IRAM fetch (~3.5us)
    # also happens during the DMA phase instead of before the first real matmul
    warm = const.tile([1, 2], F32)
    nc.scalar.activation(out=warm[:], in_=ident[0:1, 0:2].bitcast(F32),
                         func=mybir.ActivationFunctionType.Exp, scale=1.0)

    # ---- load weights (GpSimd SWDGE, parallel with the sync-queue loads;
    # wf first since the Wf^T -> W2 chain is the longest) ----
    wf_sb = const.tile([D, C], F32R)
    nc.gpsimd.dma_start(wf_sb[:], wf_d.ap())
    wq_sb = const.tile([128, NW, D], F32R)
    wk_sb = const.tile([128, NW, D], F32R)
    wv_sb = const.tile([128, NW, D], F32R)
    for w_d, w_sb in ((wq_d, wq_sb), (wk_d, wk_sb), (wv_d, wv_sb)):
        nc.gpsimd.dma_start(w_sb[:], w_d.ap().rearrange("(n p) d -> p n d", p=128))

    # persistent ping-pong tiles for the finalize stage, with row 65
    # pre-zeroed once here so no DMA sits on the finalize critical path
    # (rows are padded to 66 partitions: fp32r transposes need an even
    # innermost free count on the destination)
    ot_tiles = [const.tile([D + 2, 512], F32R, name=f"ot{i}") for i in range(2)]
    for i in range(2):
        nc.gpsimd.dma_start(ot_tiles[i][D + 1 : D + 2, :], zeros_d.ap())

    # bias broadcast to all partitions
    b_bcast = const.tile([128, D], F32)
    b_ap = b_d.ap()
    b_src = bass.AP(tensor=b_ap.tensor, offset=b_ap.offset, ap=[[0, 128]] + list(b_ap.ap))
    nc.gpsimd.dma_start(b_bcast[:], b_src)

    # x^T / y^T duplicated on partitions 64-127 for score row-packing
    xTd = const.tile([128, S], F32R)
    yTd = const.tile([128, S], F32R)
    z_sb = const.tile([128, NCHUNK, D + 1], F32R)  # z = x @ W2, col 64 = ones
    nc.gpsimd.dma_start(z_sb[:, :, D : D + 1], ones_d.ap().unsqueeze(2))

    a_sb = const.tile([D, D], F32R)        # A = Wq^T Wk
    w2_sb = const.tile([D, D], F32R)       # W2 = Wv^T Wf^T
    wfT_sb = const.tile([128, NW, D], F32R)

    # Single prep PSUM pool; phases that never overlap share tags so the
    # total stays within 7 banks. Emission order puts the x^T transposes
    # first — they gate the whole main loop.
    with tc.tile_pool(name="prep_ps", bufs=1, space="PSUM") as pps:
        pe_warm = pps.tile([32, 32], F32R, tag="warm", bufs=1)
        nc.tensor.transpose(pe_warm[:], ident[0:32, 0:32], ident[0:32, 0:32])

        # x^T via PE transposes (copies alternate Vector/Scalar); one dup
        # DMA at the end replicates rows 0-63 onto partitions 64-127
        for n in range(NCHUNK):
            pt = pps.tile([D, 128], F32R, tag="t0", bufs=3)
            nc.tensor.transpose(pt[:], x_sb[:, n, :], ident[:])
            if n % 2 == 0:
                nc.vector.tensor_copy(xTd[0:D, n * 128 : (n + 1) * 128], pt[:])
            else:
                nc.scalar.copy(xTd[0:D, n * 128 : (n + 1) * 128], pt[:])
        nc.sync.dma_start(xTd[D:128, :], xTd[0:D, :])

        # Wf^T chunks via PE transpose
        for n in range(NW):
            pt = pps.tile([128, D], F32R, tag="t1", bufs=2)
            nc.tensor.transpose(pt[:], wf_sb[:, n * 128 : (n + 1) * 128], ident[0:D, 0:D])
            nc.vector.tensor_copy(wfT_sb[:, n, :], pt[:])

        # A = Wq^T Wk (accumulate over 6 chunks of the contraction dim C)
        a_ps = pps.tile([D, D], F32, tag="t2", bufs=2)
        for n in range(NW):
            nc.tensor.matmul(a_ps[:], wq_sb[:, n, :], wk_sb[:, n, :],
                             start=(n == 0), stop=(n == NW - 1))
        nc.vector.tensor_copy(a_sb[:], a_ps[:])

        # y^T = A^T @ x^T ; rows 64-127 by one dup DMA at the end
        for j in range(NSUP):
            yp = pps.tile([D, 512], F32, tag="t2", bufs=2)
            nc.tensor.matmul(yp[:], a_sb[:], xTd[0:D, j * 512 : (j + 1) * 512],
                             start=True, stop=True)
            if j % 2 == 0:
                nc.vector.tensor_copy(yTd[0:D, j * 512 : (j + 1) * 512], yp[:])
            else:
                nc.scalar.copy(yTd[0:D, j * 512 : (j + 1) * 512], yp[:])
        nc.sync.dma_start(yTd[D:128, :], yTd[0:D, :])

        # W2 = Wv^T Wf^T
        w2_ps = pps.tile([D, D], F32, tag="t2", bufs=2)
        for n in range(NW):
            nc.tensor.matmul(w2_ps[:], wv_sb[:, n, :], wfT_sb[:, n, :],
                             start=(n == 0), stop=(n == NW - 1))
        nc.vector.tensor_copy(w2_sb[:], w2_ps[:])

        w2d = const.tile([128, D], F32R)
        nc.sync.dma_start(w2d[D:128, :], w2_sb[:])

    # ---- main loop: row-packed scores^T -> exp -> O' accumulation.
    # The z matmuls (z = x @ W2, row-packed pairs) borrow the future O'
    # accumulator banks as scratch (same pool tags) and are emitted
    # interleaved with the first pair's scores so they fill PE gaps while
    # the exp pipeline spins up instead of delaying it.
    with tc.tile_pool(name="oacc", bufs=1, space="PSUM") as oacc_pool:

        def z_pair(h):
            n0, n1 = 2 * h, 2 * h + 1
            zp0 = oacc_pool.tile([128, D], F32, tag=f"o{n0 % 4}", name=f"zp{n0}")
            zp1 = oacc_pool.tile([128, D], F32, tag=f"o{n1 % 4}", name=f"zp{n1}")
            nc.tensor.matmul(zp0[:], xTd[0:D, n0 * 128 : (n0 + 1) * 128], w2_sb[:],
                             start=True, stop=True)
            nc.tensor.matmul(zp1[:], xTd[D:128, n1 * 128 : (n1 + 1) * 128],
                             w2d[D:128, :], start=True, stop=True)
            nc.vector.tensor_copy(z_sb[:, n0, 0:D], zp0[:])
            nc.scalar.copy(z_sb[:, n1, 0:D], zp1[:])

        with tc.tile_pool(name="sc_ps", bufs=2, space="PSUM") as scp, \
             tc.tile_pool(name="et", bufs=2) as etp:

            def scores_exp(p):
                n0, n1 = 2 * p, 2 * p + 1
                lhs0 = xTd[0:D, n0 * 128 : (n0 + 1) * 128]
                lhs1 = xTd[D:128, n1 * 128 : (n1 + 1) * 128]
                # eT layout: [j-block 4][chunk 2][512]
                eT = etp.tile([128, 2 * S], F32R, tag="et", name=f"eT{p}")
                for j in range(NSUP):
                    sc = scp.tile([128, 1024], F32, tag="sc", name=f"sc{p}_{j}")
                    # two K=64 matmuls packed into PE row groups 0-1 / 2-3
                    nc.tensor.matmul(sc[:, 0:512], lhs0,
                                     yTd[0:D, j * 512 : (j + 1) * 512],
                                     start=True, stop=True)
                    nc.tensor.matmul(sc[:, 512:1024], lhs1,
                                     yTd[D:128, j * 512 : (j + 1) * 512],
                                     start=True, stop=True)
                    nc.scalar.activation(out=eT[:, j * 1024 : (j + 1) * 1024], in_=sc[:],
                                         func=mybir.ActivationFunctionType.Exp,
                                         scale=SCALING)
                return eT

            def oprime(p, eT, o_ps):
                n0, n1 = 2 * p, 2 * p + 1
                for j in range(NSUP):
                    nc.tensor.matmul(o_ps[j][:], z_sb[:, n0, :],
                                     eT[:, j * 1024 : j * 1024 + 512],
                                     start=(p == 0), stop=False)
                    nc.tensor.matmul(o_ps[j][:], z_sb[:, n1, :],
                                     eT[:, j * 1024 + 512 : (j + 1) * 1024],
                                     start=False, stop=(p == NPAIR - 1))

            for h in range(4):
                z_pair(h)
            eT0 = scores_exp(0)
            for h in range(4, NPAIR):
                z_pair(h)
            o_ps = [oacc_pool.tile([D + 1, 512], F32, tag=f"o{j}", name=f"o_ps{j}")
                    for j in range(NSUP)]
            oprime(0, eT0, o_ps)
            for p in range(1, NPAIR):
                eT = scores_exp(p)
                oprime(p, eT, o_ps)

        # ---- finalize: transpose back, normalize, bias, store.
        # All 4 query-128-chunks of a superblock land in ONE psum tile so the
        # reciprocal / multiply / bias-add run as single wide ops (DVE DRAIN
        # overhead is per-instruction, so fewer, fatter ops win).
        with tc.tile_pool(name="fin_ps", bufs=2, space="PSUM") as fps, \
             tc.tile_pool(name="fin_sb", bufs=2) as fsb, \
             tc.tile_pool(name="out_sb", bufs=2) as osb:
            out_r = out_d.ap().rearrange("(j q p) d -> j p q d", p=128, q=4)
            for j in range(NSUP):
                # ping-pong const tiles whose padding row 65 is pre-zeroed;
                # copy split between Vector and Scalar by column halves
                ot = ot_tiles[j % 2]
                nc.vector.tensor_copy(ot[0 : D + 1, 0:256], o_ps[j][:, 0:256])
                nc.scalar.copy(ot[0 : D + 1, 256:512], o_ps[j][:, 256:512])
                pt = fps.tile([128, 4, D + 2], F32R, tag="fin")
                for q in range(4):
                    nc.tensor.transpose(pt[:, q, :], ot[:, q * 128 : (q + 1) * 128],
                                        ident[0 : D + 2, 0 : D + 2])
                r_sb = fsb.tile([128, 4], F32, tag="r")
                nc.vector.reciprocal(r_sb[:], pt[:, :, D : D + 1].bitcast(F32))
                o_out = osb.tile([128, 4, D], F32, tag="oo")
                nc.vector.tensor_mul(o_out[:], pt[:, :, 0:D],
                                     r_sb[:].unsqueeze(2).broadcast_to([128, 4, D]))
                nc.gpsimd.tensor_add(
                    o_out[:], o_out[:],
                    b_bcast[:].unsqueeze(1).broadcast_to([128, 4, D]))
                nc.sync.dma_start(out_r[j], o_out[:])

    const.release()


_NC_CACHE = {}


def _get_nc():
    if "nc" not in _NC_CACHE:
        _NC_CACHE["nc"] = _build_nc()
    return _NC_CACHE["nc"]


def kernel(x, w_q, w_k, w_v, w_final, b_final, _trace=False):
    nc = _get_nc()
    x = np.ascontiguousarray(np.asarray(x, dtype=np.float32))
    shared = {
        "w_q": np.ascontiguousarray(np.asarray(w_q, dtype=np.float32)),
        "w_k": np.ascontiguousarray(np.asarray(w_k, dtype=np.float32)),
        "w_v": np.ascontiguousarray(np.asarray(w_v, dtype=np.float32)),
        "w_final": np.ascontiguousarray(np.asarray(w_final, dtype=np.float32)),
        "b_final": np.ascontiguousarray(np.asarray(b_final, dtype=np.float32)),
        "ident": np.eye(128, dtype=np.float32),
        "ones": np.ones((128, NCHUNK), dtype=np.float32),
        "zeros": np.zeros((1, 512), dtype=np.float32),
    }
    in_maps = [dict(shared, x=x[b]) for b in range(B)]
    res = run_bass_kernel_spmd(nc, in_maps, core_ids=list(range(B)), trace=_trace)
    out = np.stack([res.results[b]["out"] for b in range(B)], axis=0)
    if _trace:
        return out, res
    return out
